# revision 30
# baseline (speedup 1.0000x reference)
"""Trainium2 Bass kernel for the one-hot Conv2DProduct.

Math: the reference is a VALID conv, stride (2,2), kernel 2x2, with a one-hot
HWIO weight where output channel o selects input channel (o // 32**k) % 32 at
kernel cell k (row-major cells).  With C_OUT = 512 < 32**2, cells 2 and 3
always select channel 0, so

  out[b, i, j, o] = x[b, 2i, 2j,   o % 32]      (cell 0: even row, even col)
                  + x[b, 2i, 2j+1, o // 32]     (cell 1: even row, odd col; o//32 < 16)
                  + x[b, 2i+1, 2j,   0]         (cell 2)
                  + x[b, 2i+1, 2j+1, 0]         (cell 3)

The kernel is DMA-bound in fp32 (67 MB of output per core), so it computes
and stores fp16 (rel err ~8e-4, tolerance 2e-2) and converts back to fp32 on
the host, halving store traffic to 33.6 MB/core.  The host packs, per output
pixel, 32 A-channels plus the 16 values bs[c1] = B[c1] + odd0 + odd1
(pre-summed in fp32 on the host), 48 fp16 floats per pixel.

Per core (8 batches, SBUF partition p = (batch, out row i), 4 groups of 128):

  bs2[p, j, c1, d] = xp[p, j, 32 + c1]  for d in {0, 1}   (ACT copy, tiny)
  out[p, j, c1, c0h, c0l] = xp[p, j, 2*c0h + c0l] + bs2[p, j, c1, c0l]

The main add iterates (j, c1, c0h:16, c0l:2).  Duplicating each bs value
twice (bs2) gives BOTH operands a packed 2-byte last dim (the broadcasts sit
on middle dims, which the DVE fast path tolerates), so the whole output runs
in the DVE 2x performance mode: ~0.56 ns/elem vs 1.08 for the naive
broadcast add whose in1 has a stride-0 last dim.  PACK = 50 = 25*2 factors
so the A-operand (c0h, c0l) view falls out of a plain rearrange.  DVE does
~20 us per group against the ~22 us store-DMA period; ACT only copies bs2
(~2 us).  GpSimd compute is avoided entirely (its software Add contends on
SBUF and slows every other engine; its SWDGE loads also pin to DMA engine 15
making it the straggler).  Input loads are issued on the ACT HWDGE ring and
output stores on the SP ring so prefetches never queue behind stores.
Data-parallel over batch across the 8 cores.
"""

import sys

import numpy as np

_REPO = "/opt/trn_rl_repo"
if _REPO not in sys.path:
    sys.path.insert(0, _REPO)

import concourse.bacc as bacc
import concourse.mybir as mybir
from concourse import tile
from concourse.bass_utils import run_bass_kernel_spmd

B, H, W, C = 64, 128, 128, 32
OH, OW, CO = 64, 64, 512
N_CORES = 8
B_LOC = B // N_CORES  # batches per core
F16 = mybir.dt.float16
PACK = 48  # floats per output pixel: 32 A-channels + 16 host-precomputed bs


def pack_inputs(x_local):
    """[b, H, W, C] fp32 -> xp [b, OH, OW*PACK] fp16.

    Per output pixel: 32 A-channels (even row, even col) and the 16 values
    bs[c1] = x[2i, 2j+1, c1] + x[2i+1, 2j, 0] + x[2i+1, 2j+1, 0], summed on
    the host in fp32 (one fp16 rounding instead of a device-side fp16 chain).
    """
    b = x_local.shape[0]
    ev = x_local[:, 0::2].reshape(b, OH, OW, 2, C)
    odd = x_local[:, 1::2, :, 0].reshape(b, OH, OW, 2)
    bs = ev[:, :, :, 1, :16] + odd.sum(axis=-1, keepdims=True)
    xp = np.concatenate([ev[:, :, :, 0, :], bs], axis=-1)
    return np.ascontiguousarray(xp.reshape(b, OH, OW * PACK).astype(np.float16))


def build_bass(b_loc: int = B_LOC):
    nc = bacc.Bacc("TRN2", target_bir_lowering=False, debug=False)
    xp_d = nc.dram_tensor("xp", [b_loc, OH, OW * PACK], F16, kind="ExternalInput")
    out = nc.dram_tensor("out", [b_loc, OH, OW, CO], F16, kind="ExternalOutput")

    # Chunk widths per group; group 0 ramps up so the store stream starts
    # early, the last group ramps down so the final store drains sooner.
    SCHED_N = [8] * 8
    SCHED_0 = [2, 2, 4, 8, 8, 8, 8, 8, 8, 8]
    SCHED_L = [8] * 7 + [4, 2, 2]

    with tile.TileContext(nc) as tc:
        with (
            tc.tile_pool(name="io", bufs=2) as io_pool,
            tc.tile_pool(name="mid", bufs=2) as mid_pool,
            tc.tile_pool(name="outp", bufs=10) as out_pool,
        ):
            xp_r_d = xp_d[:].rearrange("b i f -> (b i) f")
            out_d = out[:].rearrange("b i j o -> (b i) (j o)")

            n_bg = (b_loc * OH) // 128  # groups of 128 partitions
            xp_tiles = {0: io_pool.tile([128, OW * PACK], F16, name="xp0", tag="xp")}
            nc.scalar.dma_start(xp_tiles[0][:], xp_r_d[0:128, :])

            for bg in range(n_bg):
                psl = slice(bg * 128, (bg + 1) * 128)
                xp_t = xp_tiles.pop(bg)
                xp_3 = xp_t.rearrange("p (j c) -> p j c", c=PACK)
                # 48 = 24*2: halves a<16 of each 48-float row are the 32
                # A-channels as (c0h, c0l)
                xp_5 = xp_t.rearrange("p (j a d) -> p j a d", a=PACK // 2, d=2)
                bs2 = mid_pool.tile([128, OW * 32], F16, name=f"b2{bg}", tag="b2")
                bs2_4 = bs2.rearrange("p (j c1 d) -> p j c1 d", c1=16, d=2)

                if bg + 1 < n_bg:
                    nsl = slice((bg + 1) * 128, (bg + 2) * 128)
                    xp_tiles[bg + 1] = io_pool.tile(
                        [128, OW * PACK], F16, name=f"xp{bg + 1}", tag="xp"
                    )
                    nc.scalar.dma_start(xp_tiles[bg + 1][:], xp_r_d[nsl, :])

                sched = (SCHED_0 if bg == 0
                         else SCHED_L if bg == n_bg - 1 else SCHED_N)
                j0 = 0
                for ci, jw in enumerate(sched):
                    jsl = slice(j0, j0 + jw)
                    # per-chunk bs2 duplication (tiny ACT copy; per-chunk so
                    # the first add isn't gated on a whole-group copy)
                    nc.scalar.activation(
                        out=bs2_4[:, jsl, :, :],
                        in_=xp_3[:, jsl, C:C + 16].unsqueeze(3).to_broadcast(
                            [128, jw, 16, 2]
                        ),
                        func=mybir.ActivationFunctionType.Copy,
                    )
                    a_bc = xp_5[:, jsl, 0:16, :].unsqueeze(2).to_broadcast(
                        [128, jw, 16, 16, 2]
                    )
                    b_bc = bs2_4[:, jsl, :, :].unsqueeze(3).to_broadcast(
                        [128, jw, 16, 16, 2]
                    )
                    ot = out_pool.tile([128, 8 * CO], F16, name=f"ot{bg}_{ci}", tag="ot")
                    nc.vector.tensor_tensor(
                        out=ot[:, 0:jw * CO], in0=a_bc, in1=b_bc,
                        op=mybir.AluOpType.add,
                    )
                    nc.sync.dma_start(
                        out_d[psl, j0 * CO:(j0 + jw) * CO], ot[:, 0:jw * CO]
                    )
                    j0 += jw
    return nc


_NC = None


def _get_nc():
    global _NC
    if _NC is None:
        _NC = build_bass()
        _NC.compile()  # bacc register allocation + lowering
    return _NC


def kernel(**inputs):
    x = np.ascontiguousarray(np.asarray(inputs["x"], dtype=np.float32))
    assert x.shape == (B, H, W, C), x.shape
    nc = _get_nc()
    in_maps = []
    for c in range(N_CORES):
        in_maps.append({"xp": pack_inputs(x[c * B_LOC:(c + 1) * B_LOC])})
    res = run_bass_kernel_spmd(nc, in_maps, list(range(N_CORES))).results
    return np.concatenate([np.asarray(r["out"]) for r in res], axis=0).astype(
        np.float32
    )


# revision 32
# speedup vs baseline: 1.0877x; 1.0877x over previous
"""Trainium2 Bass kernel for the one-hot Conv2DProduct.

Math: the reference is a VALID conv, stride (2,2), kernel 2x2, with a one-hot
HWIO weight where output channel o selects input channel (o // 32**k) % 32 at
kernel cell k (row-major cells).  With C_OUT = 512 < 32**2, cells 2 and 3
always select channel 0, so

  out[b, i, j, o] = x[b, 2i, 2j,   o % 32]      (cell 0: even row, even col)
                  + x[b, 2i, 2j+1, o // 32]     (cell 1: even row, odd col; o//32 < 16)
                  + x[b, 2i+1, 2j,   0]         (cell 2)
                  + x[b, 2i+1, 2j+1, 0]         (cell 3)

The kernel is DMA-bound in fp32 (67 MB of output per core), so it computes
and stores fp16 (rel err ~8e-4, tolerance 2e-2) and converts back to fp32 on
the host, halving store traffic to 33.6 MB/core.  The host packs, per output
pixel, 32 A-channels plus the 16 values bs[c1] = B[c1] + odd0 + odd1
(pre-summed in fp32 on the host), 48 fp16 floats per pixel.

Per core (8 batches, SBUF partition p = (batch, out row i), 4 groups of 128):

  bs2[p, j, c1, d] = xp[p, j, 32 + c1]  for d in {0, 1}   (ACT copy, tiny)
  out[p, j, c1, c0h, c0l] = xp[p, j, 2*c0h + c0l] + bs2[p, j, c1, c0l]

The main add iterates (j, c1, c0h:16, c0l:2).  Duplicating each bs value
twice (bs2) gives BOTH operands a packed 2-byte last dim (the broadcasts sit
on middle dims, which the DVE fast path tolerates), so the whole output runs
in the DVE 2x performance mode: ~0.56 ns/elem vs 1.08 for the naive
broadcast add whose in1 has a stride-0 last dim.  PACK = 50 = 25*2 factors
so the A-operand (c0h, c0l) view falls out of a plain rearrange.  DVE does
~20 us per group against the ~22 us store-DMA period; ACT only copies bs2
(~2 us).  GpSimd compute is avoided entirely (its software Add contends on
SBUF and slows every other engine; its SWDGE loads also pin to DMA engine 15
making it the straggler).  Input loads are issued on the ACT HWDGE ring and
output stores on the SP ring so prefetches never queue behind stores.
Data-parallel over batch across the 8 cores.
"""

import sys

import numpy as np

_REPO = "/opt/trn_rl_repo"
if _REPO not in sys.path:
    sys.path.insert(0, _REPO)

import concourse.bacc as bacc
import concourse.mybir as mybir
from concourse import tile
from concourse.bass_utils import run_bass_kernel_spmd

B, H, W, C = 64, 128, 128, 32
OH, OW, CO = 64, 64, 512
N_CORES = 8
B_LOC = B // N_CORES  # batches per core
F16 = mybir.dt.float16
PACK = 48  # floats per output pixel: 32 A-channels + 16 host-precomputed bs


def pack_inputs(x_local):
    """[b, H, W, C] fp32 -> xp [b, OH, OW*PACK] fp16.

    Per output pixel: 32 A-channels (even row, even col) and the 16 values
    bs[c1] = x[2i, 2j+1, c1] + x[2i+1, 2j, 0] + x[2i+1, 2j+1, 0], summed on
    the host in fp32 (one fp16 rounding instead of a device-side fp16 chain).
    """
    b = x_local.shape[0]
    ev = x_local[:, 0::2].reshape(b, OH, OW, 2, C)
    odd = x_local[:, 1::2, :, 0].reshape(b, OH, OW, 2)
    bs = ev[:, :, :, 1, :16] + odd.sum(axis=-1, keepdims=True)
    xp = np.concatenate([ev[:, :, :, 0, :], bs], axis=-1)
    return np.ascontiguousarray(xp.reshape(b, OH, OW * PACK).astype(np.float16))


def build_bass(b_loc: int = B_LOC):
    nc = bacc.Bacc("TRN2", target_bir_lowering=False, debug=False)
    xp_d = nc.dram_tensor("xp", [b_loc, OH, OW * PACK], F16, kind="ExternalInput")
    out = nc.dram_tensor("out", [b_loc, OH, OW, CO], F16, kind="ExternalOutput")

    # Chunk widths per group; group 0 ramps up so the store stream starts
    # early, the last group ramps down so the final store drains sooner.
    SCHED_N = [8] * 8
    SCHED_0 = [2, 2, 4, 8, 8, 8, 8, 8, 8, 8]
    SCHED_L = [8] * 7 + [4, 2, 2]

    with tile.TileContext(nc) as tc:
        with (
            tc.tile_pool(name="io", bufs=2) as io_pool,
            tc.tile_pool(name="mid", bufs=2) as mid_pool,
            tc.tile_pool(name="outp", bufs=10) as out_pool,
        ):
            xp_r_d = xp_d[:].rearrange("b i f -> (b i) f")
            out_d = out[:].rearrange("b i j o -> (b i) (j o)")

            n_bg = (b_loc * OH) // 128  # groups of 128 partitions
            xp_tiles = {0: io_pool.tile([128, OW * PACK], F16, name="xp0", tag="xp")}
            nc.scalar.dma_start(xp_tiles[0][:], xp_r_d[0:128, :])

            for bg in range(n_bg):
                psl = slice(bg * 128, (bg + 1) * 128)
                xp_t = xp_tiles.pop(bg)
                xp_3 = xp_t.rearrange("p (j c) -> p j c", c=PACK)
                # 48 = 24*2: halves a<16 of each 48-float row are the 32
                # A-channels as (c0h, c0l)
                xp_5 = xp_t.rearrange("p (j a d) -> p j a d", a=PACK // 2, d=2)
                bs2 = mid_pool.tile([128, OW * 32], F16, name=f"b2{bg}", tag="b2")
                bs2_4 = bs2.rearrange("p (j c1 d) -> p j c1 d", c1=16, d=2)
                nc.scalar.activation(
                    out=bs2_4[:, :, :, :],
                    in_=xp_3[:, :, C:C + 16].unsqueeze(3).to_broadcast(
                        [128, OW, 16, 2]
                    ),
                    func=mybir.ActivationFunctionType.Copy,
                )

                if bg + 1 < n_bg:
                    nsl = slice((bg + 1) * 128, (bg + 2) * 128)
                    xp_tiles[bg + 1] = io_pool.tile(
                        [128, OW * PACK], F16, name=f"xp{bg + 1}", tag="xp"
                    )
                    nc.scalar.dma_start(xp_tiles[bg + 1][:], xp_r_d[nsl, :])

                sched = (SCHED_0 if bg == 0
                         else SCHED_L if bg == n_bg - 1 else SCHED_N)
                j0 = 0
                for ci, jw in enumerate(sched):
                    jsl = slice(j0, j0 + jw)
                    a_bc = xp_5[:, jsl, 0:16, :].unsqueeze(2).to_broadcast(
                        [128, jw, 16, 16, 2]
                    )
                    b_bc = bs2_4[:, jsl, :, :].unsqueeze(3).to_broadcast(
                        [128, jw, 16, 16, 2]
                    )
                    ot = out_pool.tile([128, 8 * CO], F16, name=f"ot{bg}_{ci}", tag="ot")
                    nc.vector.tensor_tensor(
                        out=ot[:, 0:jw * CO], in0=a_bc, in1=b_bc,
                        op=mybir.AluOpType.add,
                    )
                    nc.sync.dma_start(
                        out_d[psl, j0 * CO:(j0 + jw) * CO], ot[:, 0:jw * CO]
                    )
                    j0 += jw
    return nc


_NC = None


def _get_nc():
    global _NC
    if _NC is None:
        _NC = build_bass()
        _NC.compile()  # bacc register allocation + lowering
    return _NC


def kernel(**inputs):
    x = np.ascontiguousarray(np.asarray(inputs["x"], dtype=np.float32))
    assert x.shape == (B, H, W, C), x.shape
    nc = _get_nc()
    in_maps = []
    for c in range(N_CORES):
        in_maps.append({"xp": pack_inputs(x[c * B_LOC:(c + 1) * B_LOC])})
    res = run_bass_kernel_spmd(nc, in_maps, list(range(N_CORES))).results
    return np.concatenate([np.asarray(r["out"]) for r in res], axis=0).astype(
        np.float32
    )
